# revision 1
# baseline (speedup 1.0000x reference)
"""HGCN decoder on 8 trn2 NeuronCores.

Strategy: nodes are sorted by in-degree, grouped into 128-node tiles, and the
tiles are dealt round-robin across the 8 cores (graph-parallel by destination
node).  Each core:
  - runs the node-wise hyperbolic math (HypLinear / exp / log maps) on its
    4096 nodes, tile by tile, with the per-node scalar chains batched into
    [128, 32] arrays,
  - publishes its tangent-space table shard, AllGathers the full [32768, 64]
    table to DRAM,
  - aggregates messages with `dma_gather` (padded per-tile CSR: tile t gathers
    [128, K_t, 64] source rows in one indirect DMA) followed by a weighted
    strided reduce on the vector engine,
  - finishes with the euclidean readout matmul.
All graph preprocessing (permutation, padded neighbor tables, weight folding
of edge/node masks) happens host-side in numpy; the device only sees dense
tables.
"""

import numpy as np

N = 32768
E = 1015808
D = 64
C = 8          # cores
NL = N // C    # 4096 nodes per core
P = 128        # partitions / tile
T = NL // P    # 32 tiles per core
MAXN = 1.0 - 4e-3   # PROJ_EPS boundary for c=1
EPS = 1e-15
ART_CLIP = 1.0 - 1e-5
MAX_TANH = 15.0


def _build_tables(rows, cols, edge_mask, node_mask):
    """Permute nodes by degree, deal tiles round-robin to cores, and build the
    per-core padded gather tables (int16 indices wrapped the way
    InstDMAGatherAnt wants them) plus matching weight tables."""
    deg = np.bincount(rows, minlength=N)
    order = np.argsort(-deg, kind="stable")
    # global tile j -> core j%C, slot j//C ; permuted position of its p-th node
    perm = np.empty(N, dtype=np.int64)
    j = np.arange(N) // P                     # global tile of sorted rank r
    c = j % C
    t = j // C
    p = np.arange(N) % P
    perm[c * NL + t * P + p] = order          # perm[g] = original node id
    pos = np.empty(N, dtype=np.int64)
    pos[perm] = np.arange(N)

    # gather-table row id for permuted position g=(c,t,p):
    #   AllGather concatenates per-core [P, T*D] blocks, so
    #   row_id = c*NL + p*T + t
    gg = np.arange(N)
    gc, gr = gg // NL, gg % NL
    gt, gp_ = gr // P, gr % P
    rowid = gc * NL + gp_ * T + gt            # [g] -> table row
    dstpos = pos[rows]
    eorder = np.argsort(dstpos, kind="stable")
    src_sorted = rowid[pos[cols[eorder]]]     # gather table rows, 0..N-1
    w_sorted = edge_mask[eorder, 0].astype(np.float64)
    cnts = np.bincount(dstpos, minlength=N)
    offs = np.zeros(N + 1, dtype=np.int64)
    np.cumsum(cnts, out=offs[1:])

    # per-slot K: max count over the 8 cores' tiles in that slot
    cnts_g = cnts.reshape(C, T, P)
    Ks = np.maximum(cnts_g.max(axis=(0, 2)), 1).astype(np.int64)   # [T]

    IDXC = int(8 * Ks.sum())
    WTC = int(Ks.sum())
    idx_dev = np.zeros((C, P, IDXC), np.int16)
    wt_dev = np.zeros((C, P, WTC), np.float32)
    nm = node_mask[:, 0].astype(np.float64)
    ioff = woff = 0
    ar = None
    for t in range(T):
        K = int(Ks[t])
        if ar is None or ar.shape[1] != K:
            ar = np.arange(K)[None, :]
        for cc in range(C):
            base = cc * NL + t * P
            cn = cnts[base:base + P]
            take = offs[base:base + P][:, None] + ar          # [P, K]
            valid = ar < cn[:, None]
            take_c = np.minimum(take, E - 1)
            nb = np.where(valid, src_sorted[take_c], 0)
            wl = np.where(valid, w_sorted[take_c], 0.0)
            wl = wl * nm[perm[base:base + P]][:, None]
            il = nb.T.reshape(-1)                             # i = g*128+p
            ch = il.reshape(8 * K, 16).T                      # [16, 8K]
            idx_dev[cc, :, ioff:ioff + 8 * K] = np.tile(ch, (8, 1)).astype(np.int16)
            wt_dev[cc, :, woff:woff + K] = wl.astype(np.float32)
        ioff += 8 * K
        woff += K
    # pad counts per (core, slot, partition) for the pad-subtract path
    pc_dev = np.zeros((C, 1, T * P), np.float32)
    for t in range(T):
        K = int(Ks[t])
        for cc in range(C):
            base = cc * NL + t * P
            pc_dev[cc, 0, t * P:(t + 1) * P] = K - cnts[base:base + P]
    allones = bool(np.all(edge_mask == 1.0) and np.all(node_mask == 1.0))
    return perm, Ks, idx_dev, wt_dev, IDXC, WTC, pc_dev, allones


def _build_program(Ks, IDXC, WTC, use_wt=True, sim=False):
    import os
    import concourse.bacc as bacc
    import concourse.bass as bass
    import concourse.mybir as mybir
    import concourse.tile as tile
    from concourse import library_config
    from concourse.masks import make_identity

    f32 = mybir.dt.float32
    i16 = mybir.dt.int16
    AF = mybir.ActivationFunctionType
    OP = mybir.AluOpType
    X = mybir.AxisListType.X

    nc = bacc.Bacc("TRN2", target_bir_lowering=False, debug=False,
                   num_devices=1 if sim else C)
    ablate = set(os.environ.get("KABLATE", "").split(",")) if sim else set()

    h_in = nc.dram_tensor("h_in", [P, T * D], f32, kind="ExternalInput")
    idx_in = nc.dram_tensor("idx_in", [P, IDXC], i16, kind="ExternalInput")
    wt_in = nc.dram_tensor("wt_in", [P, WTC], f32, kind="ExternalInput")
    w0t_in = nc.dram_tensor("w0t_in", [D, D], f32, kind="ExternalInput")
    w1t_in = nc.dram_tensor("w1t_in", [D, D], f32, kind="ExternalInput")
    wot_in = nc.dram_tensor("wot_in", [D, 16], f32, kind="ExternalInput")
    pc_in = nc.dram_tensor("pc_in", [1, T * P], f32, kind="ExternalInput")
    out_dram = nc.dram_tensor("out", [P, T * 16], f32, kind="ExternalOutput")
    xt_shard = nc.dram_tensor("xt_shard", [P, T * D], f32)
    xt_table = nc.dram_tensor("xt_table", [N, D], f32, addr_space="Shared")
    groups = [list(range(C))]

    with tile.TileContext(nc) as tc:
        nc.gpsimd.load_library(library_config.mlp)
        import contextlib
        ctx = contextlib.ExitStack()
        with ctx:
            const = ctx.enter_context(tc.tile_pool(name="const", bufs=1))
            sqp = ctx.enter_context(tc.tile_pool(name="sq", bufs=3))
            xtp = ctx.enter_context(tc.tile_pool(name="xtp", bufs=3))
            gp = ctx.enter_context(tc.tile_pool(name="gp", bufs=3))
            scp = ctx.enter_context(tc.tile_pool(name="scp", bufs=2))
            psp = ctx.enter_context(tc.tile_pool(name="psp", bufs=2, space="PSUM"))
            psmv = ctx.enter_context(tc.tile_pool(name="psmv", bufs=2, space="PSUM"))

            ident = const.tile([P, P], f32)
            make_identity(nc, ident[:])
            idx_sb = const.tile([P, IDXC], i16)
            nc.sync.dma_start(out=idx_sb[:], in_=idx_in[:])
            wt_sb = const.tile([P, WTC], f32)
            nc.sync.dma_start(out=wt_sb[:], in_=wt_in[:])
            w0t_sb = const.tile([D, D], f32)
            nc.sync.dma_start(out=w0t_sb[:], in_=w0t_in[:])
            w1t_sb = const.tile([D, D], f32)
            nc.sync.dma_start(out=w1t_sb[:], in_=w1t_in[:])
            wot_sb = const.tile([D, 16], f32)
            nc.sync.dma_start(out=wot_sb[:], in_=wot_in[:])
            pc_sb = const.tile([1, T * P], f32)
            nc.sync.dma_start(out=pc_sb[:], in_=pc_in[:])

            x_sb = const.tile([P, T * D], f32)      # node state (manifold)
            mv_sb = const.tile([P, T * D], f32)     # W@x then xt (tangent msgs)
            agg_sb = const.tile([P, T * D], f32)    # aggregated tangent
            u_sb = const.tile([P, T * D], f32)      # relu'd tangent
            out_sb = const.tile([P, T * 16], f32)

            nc.sync.dma_start(out=x_sb[:], in_=h_in[:])

            def ts(t, w=D):
                return slice(t * w, (t + 1) * w)

            def artanh(dst, src):
                """dst = 0.5*ln((1+c)/(1-c)), c = min(src, ART_CLIP); src>=0."""
                cth = scp.tile([P, T], f32, tag="art_c")
                nc.vector.tensor_scalar_min(cth[:], src[:], ART_CLIP)
                pt = scp.tile([P, T], f32, tag="art_p")
                nc.scalar.activation(pt[:], cth[:], AF.Copy, bias=1.0)
                mt = scp.tile([P, T], f32, tag="art_m")
                nc.scalar.activation(mt[:], cth[:], AF.Copy, scale=-1.0, bias=1.0)
                rm = scp.tile([P, T], f32, tag="art_rm")
                nc.vector.reciprocal(rm[:], mt[:])
                nc.vector.tensor_tensor(pt[:], pt[:], rm[:], op=OP.mult)
                nc.scalar.activation(pt[:], pt[:], AF.Ln)
                nc.vector.tensor_scalar_mul(dst[:], pt[:], 0.5)

            def norm_from_sq(dst, src):
                nc.scalar.activation(dst[:], src[:], AF.Sqrt)
                nc.vector.tensor_scalar_max(dst[:], dst[:], EPS)

            def exp_proj_scale(dst, nrm):
                """dst = min(tanh(min(nrm,15)), MAXN) / nrm"""
                a = scp.tile([P, T], f32, tag="eps_a")
                nc.vector.tensor_scalar_min(a[:], nrm[:], MAX_TANH)
                nc.scalar.activation(a[:], a[:], AF.Tanh)
                nc.vector.tensor_scalar_min(a[:], a[:], MAXN)
                r = scp.tile([P, T], f32, tag="eps_r")
                nc.vector.reciprocal(r[:], nrm[:])
                nc.vector.tensor_tensor(dst[:], a[:], r[:], op=OP.mult)

            # ---- x0 = proj(expmap0(h)) --------------------------------------
            nh2 = scp.tile([P, T], f32, tag="nh2")
            for t in range(T):
                sq = sqp.tile([P, D], f32, tag="sq")
                nc.scalar.activation(sq[:], x_sb[:, ts(t)], AF.Square,
                                     accum_out=nh2[:, t:t + 1])
            nh = scp.tile([P, T], f32, tag="nh")
            norm_from_sq(nh, nh2)
            s0 = scp.tile([P, T], f32, tag="s0")
            exp_proj_scale(s0, nh)
            for t in range(T):
                nc.vector.tensor_scalar_mul(x_sb[:, ts(t)], x_sb[:, ts(t)],
                                            s0[:, t:t + 1])

            for layer in range(2):
                w_l = w0t_sb if layer == 0 else w1t_sb
                # ---- HypLinear + logmap0 (analytic combined scale) ----------
                xn2 = scp.tile([P, T], f32, tag="xn2")
                mxn2 = scp.tile([P, T], f32, tag="mxn2")
                for t in range(T):
                    sq = sqp.tile([P, D], f32, tag="sq")
                    nc.scalar.activation(sq[:], x_sb[:, ts(t)], AF.Square,
                                         accum_out=xn2[:, t:t + 1])
                    xT_ps = psp.tile([D, P], f32, tag="xT")
                    nc.tensor.transpose(out=xT_ps[:], in_=x_sb[:, ts(t)],
                                        identity=ident[:])
                    xT = xtp.tile([D, P], f32, tag="xT_sb")
                    nc.vector.tensor_copy(xT[:], xT_ps[:])
                    mv_ps = psmv.tile([P, D], f32, tag="mv")
                    nc.tensor.matmul(out=mv_ps[:], lhsT=xT[:], rhs=w_l[:],
                                     start=True, stop=True)
                    sq2 = sqp.tile([P, D], f32, tag="sq")
                    nc.scalar.activation(sq2[:], mv_ps[:], AF.Square,
                                         accum_out=mxn2[:, t:t + 1])
                    nc.vector.tensor_copy(mv_sb[:, ts(t)], mv_ps[:])
                xn = scp.tile([P, T], f32, tag="xn")
                norm_from_sq(xn, xn2)
                mxn = scp.tile([P, T], f32, tag="mxn")
                norm_from_sq(mxn, mxn2)
                at = scp.tile([P, T], f32, tag="at")
                artanh(at, xn)
                rx = scp.tile([P, T], f32, tag="rx")
                nc.vector.reciprocal(rx[:], xn[:])
                nc.vector.tensor_tensor(at[:], at[:], rx[:], op=OP.mult)
                nc.vector.tensor_tensor(at[:], at[:], mxn[:], op=OP.mult)
                nc.vector.tensor_scalar_min(at[:], at[:], MAX_TANH)
                nc.scalar.activation(at[:], at[:], AF.Tanh)
                nc.vector.tensor_scalar_min(at[:], at[:], MAXN)
                smsg = scp.tile([P, T], f32, tag="smsg")
                artanh(smsg, at)
                rmx = scp.tile([P, T], f32, tag="rmx")
                nc.vector.reciprocal(rmx[:], mxn[:])
                nc.vector.tensor_tensor(smsg[:], smsg[:], rmx[:], op=OP.mult)
                for t in range(T):
                    nc.vector.tensor_scalar_mul(mv_sb[:, ts(t)], mv_sb[:, ts(t)],
                                                smsg[:, t:t + 1])
                # publish shard and AllGather the full tangent table
                nc.sync.dma_start(out=xt_shard[:], in_=mv_sb[:])
                nobar = "nobar" in ablate
                if sim:
                    if not nobar:
                        tc.strict_bb_all_engine_barrier()
                    nc.sync.dma_start(
                        out=xt_table[0:NL, :].rearrange("(p x) d -> p x d", p=P),
                        in_=xt_shard[:].rearrange("p (x d) -> p x d", d=D))
                    if not nobar:
                        tc.strict_bb_all_engine_barrier()
                else:
                    if not nobar:
                        tc.strict_bb_all_engine_barrier()
                    nc.gpsimd.collective_compute(
                        "AllGather", mybir.AluOpType.bypass, replica_groups=groups,
                        ins=[xt_shard[:, :]], outs=[xt_table[:, :]])
                    if not nobar:
                        tc.strict_bb_all_engine_barrier()

                # ---- aggregation: gather + weighted reduce ------------------
                row0_sb = scp.tile([1, D], f32, tag="row0")
                if not use_wt:
                    nc.sync.dma_start(out=row0_sb[:], in_=xt_table[0:1, :])
                na2 = scp.tile([P, T], f32, tag="na2")
                ioff = woff = 0
                for t in range(T):
                    K = int(Ks[t])
                    g = gp.tile([P, K * D], f32, tag="G")
                    g3 = g[:].rearrange("p (k d) -> p k d", d=D)
                    if "gather" not in ablate:
                        nc.gpsimd.dma_gather(
                            g3, xt_table[:, :], idx_sb[:, ioff:ioff + 8 * K],
                            num_idxs=P * K, num_idxs_reg=P * K, elem_size=D,
                            single_packet=False)
                    if use_wt and "wtmul" not in ablate:
                        wt_ap = wt_sb[:, woff:woff + K]
                        wv = bass.AP(wt_ap.tensor, wt_ap.offset,
                                     list(wt_ap.ap) + [[0, D]])
                        nc.vector.tensor_tensor(g3, g3, wv, op=OP.mult)
                    if "reduce" not in ablate:
                        nc.vector.tensor_reduce(
                            agg_sb[:, ts(t)],
                            g[:].rearrange("p (k d) -> p d k", d=D),
                            axis=X, op=OP.add)
                    if not use_wt:
                        corr_ps = psmv.tile([P, D], f32, tag="mv")
                        nc.tensor.matmul(
                            out=corr_ps[:], lhsT=pc_sb[0:1, t * P:(t + 1) * P],
                            rhs=row0_sb[0:1, :], start=True, stop=True)
                        nc.vector.tensor_tensor(agg_sb[:, ts(t)],
                                                agg_sb[:, ts(t)], corr_ps[:],
                                                op=OP.subtract)
                    sq = sqp.tile([P, D], f32, tag="sq")
                    nc.scalar.activation(sq[:], agg_sb[:, ts(t)], AF.Square,
                                         accum_out=na2[:, t:t + 1])
                    ioff += 8 * K
                    woff += K
                # s2 = artanh(min(tanh(min(na,15)),MAXN)) / na
                na = scp.tile([P, T], f32, tag="na")
                norm_from_sq(na, na2)
                a2 = scp.tile([P, T], f32, tag="a2")
                nc.vector.tensor_scalar_min(a2[:], na[:], MAX_TANH)
                nc.scalar.activation(a2[:], a2[:], AF.Tanh)
                nc.vector.tensor_scalar_min(a2[:], a2[:], MAXN)
                s2 = scp.tile([P, T], f32, tag="s2")
                artanh(s2, a2)
                rna = scp.tile([P, T], f32, tag="rna")
                nc.vector.reciprocal(rna[:], na[:])
                nc.vector.tensor_tensor(s2[:], s2[:], rna[:], op=OP.mult)
                # u = relu(agg * s2); nu2 accum
                nu2 = scp.tile([P, T], f32, tag="nu2")
                for t in range(T):
                    nc.scalar.activation(u_sb[:, ts(t)], agg_sb[:, ts(t)],
                                         AF.Relu, scale=s2[:, t:t + 1])
                    sq = sqp.tile([P, D], f32, tag="sq")
                    nc.scalar.activation(sq[:], u_sb[:, ts(t)], AF.Square,
                                         accum_out=nu2[:, t:t + 1])
                nu = scp.tile([P, T], f32, tag="nu")
                norm_from_sq(nu, nu2)
                s3 = scp.tile([P, T], f32, tag="s3")
                exp_proj_scale(s3, nu)
                for t in range(T):
                    nc.vector.tensor_scalar_mul(x_sb[:, ts(t)], u_sb[:, ts(t)],
                                                s3[:, t:t + 1])

            # ---- readout: out = x @ W_out.T (b_out == 0) --------------------
            for t in range(T):
                xT_ps = psp.tile([D, P], f32, tag="xT")
                nc.tensor.transpose(out=xT_ps[:], in_=x_sb[:, ts(t)],
                                    identity=ident[:])
                xT = xtp.tile([D, P], f32, tag="xT_sb")
                nc.vector.tensor_copy(xT[:], xT_ps[:])
                o_ps = psmv.tile([P, 16], f32, tag="mv")
                nc.tensor.matmul(out=o_ps[:], lhsT=xT[:], rhs=wot_sb[:],
                                 start=True, stop=True)
                nc.vector.tensor_copy(out_sb[:, ts(t, 16)], o_ps[:])
            nc.sync.dma_start(out=out_dram[:], in_=out_sb[:])
    nc.compile()
    return nc


def kernel(h, distances, rows, cols, node_mask, edge_mask,
           W0, b0, W1, b1, W_out, b_out, _trace=False):
    from concourse.bass_utils import run_bass_kernel_spmd

    h = np.asarray(h, dtype=np.float32)
    rows = np.asarray(rows).astype(np.int64)
    cols = np.asarray(cols).astype(np.int64)
    node_mask = np.asarray(node_mask, dtype=np.float32)
    edge_mask = np.asarray(edge_mask, dtype=np.float32)
    assert not np.any(np.asarray(b0)) and not np.any(np.asarray(b1)) and \
        not np.any(np.asarray(b_out)), "nonzero biases unsupported"

    perm, Ks, idx_dev, wt_dev, IDXC, WTC, pc_dev, allones = _build_tables(
        rows, cols, edge_mask, node_mask)

    hp = h[perm].reshape(C, T, P, D).transpose(0, 2, 1, 3).reshape(C, P, T * D)
    w0t = np.ascontiguousarray(np.asarray(W0, np.float32).T)
    w1t = np.ascontiguousarray(np.asarray(W1, np.float32).T)
    wot = np.ascontiguousarray(np.asarray(W_out, np.float32).T)

    nc = _build_program(Ks, IDXC, WTC, use_wt=not allones)
    in_maps = [{
        "h_in": np.ascontiguousarray(hp[c]),
        "idx_in": idx_dev[c],
        "wt_in": wt_dev[c],
        "w0t_in": w0t, "w1t_in": w1t, "wot_in": wot,
        "pc_in": pc_dev[c],
    } for c in range(C)]
    res = run_bass_kernel_spmd(nc, in_maps, list(range(C)), trace=_trace)
    od = np.stack([res.results[c]["out"] for c in range(C)])
    od = od.reshape(C, P, T, 16).transpose(0, 2, 1, 3).reshape(N, 16)
    out = np.empty((N, 16), np.float32)
    out[perm] = od
    if _trace:
        return out, res
    return out



# revision 19
# speedup vs baseline: 1.3167x; 1.3167x over previous
"""HGCN decoder on 8 trn2 NeuronCores.

Strategy: nodes are sorted by in-degree, grouped into 128-node tiles, and the
tiles are dealt round-robin across the 8 cores (graph-parallel by destination
node).  Per layer the kernel runs a continuous gather-DMA stream (the cost
floor) and hides everything else beneath it:
  - tiles are gathered smallest-degree-first in chunks of 8; each chunk's
    weighted reduce, hyperbolic chain math, relu, the NEXT layer's matvec
    (transpose + matmul on PE) and the final readout all ride under the
    remaining gather stream,
  - all per-node scalar chains use only {Square, Copy, Exp, Ln, Relu} so the
    whole program needs a single activation-table load (tanh/sqrt/artanh are
    rewritten in exp/ln form),
  - manifold scale factors cancel analytically through the matvec: the
    published tangent message is (artanh(at)/|Wu|) * (W^T u) with
    at = min(tanh(|Wu|/|u| * artanh(a)), 1-eps), so no explicit manifold
    tensor is ever materialized.
All graph preprocessing (permutation, padded neighbor tables, pad-count
correction) happens host-side in numpy; the device only sees dense tables.
"""

import numpy as np

N = 32768
E = 1015808
D = 64
C = 8          # cores
NL = N // C    # 4096 nodes per core
P = 128        # partitions / tile
T = NL // P    # 32 tiles per core
MAXN = 1.0 - 4e-3   # PROJ_EPS boundary for c=1
ARTMAXN = 3.106303047875759   # artanh(MAXN)
IART2 = 1.0 / (ARTMAXN * ARTMAXN)
EPS = 1e-15
MAX_TANH = 15.0
# contiguous slot chunks, streamed smallest-K (highest slot) first; the last
# two are half-size to shrink the exposed trailing compute at layer edges
CHUNKS = ((24, 8), (16, 8), (8, 8), (4, 4), (0, 4))


def _build_tables(rows, cols, edge_mask, node_mask):
    """Permute nodes by degree, deal tiles round-robin to cores, and build the
    per-core padded gather tables (int16 indices wrapped the way
    InstDMAGatherAnt wants them) plus matching weight tables."""
    deg = np.bincount(rows, minlength=N)
    order = np.argsort(-deg, kind="stable")
    # global tile j -> core j%C, slot j//C ; permuted position of its p-th node
    perm = np.empty(N, dtype=np.int64)
    j = np.arange(N) // P                     # global tile of sorted rank r
    c = j % C
    t = j // C
    p = np.arange(N) % P
    perm[c * NL + t * P + p] = order          # perm[g] = original node id
    pos = np.empty(N, dtype=np.int64)
    pos[perm] = np.arange(N)

    # gather-table row id for permuted position g=(c,t,p):
    #   AllGather concatenates per-core [P, T*D] blocks, so
    #   row_id = c*NL + p*T + t
    gg = np.arange(N)
    gc, gr = gg // NL, gg % NL
    gt, gp_ = gr // P, gr % P
    rowid = gc * NL + gp_ * T + gt            # [g] -> table row
    dstpos = pos[rows]
    eorder = np.argsort(dstpos, kind="stable")
    src_sorted = rowid[pos[cols[eorder]]]     # gather table rows, 0..N-1
    w_sorted = edge_mask[eorder, 0].astype(np.float64)
    cnts = np.bincount(dstpos, minlength=N)
    offs = np.zeros(N + 1, dtype=np.int64)
    np.cumsum(cnts, out=offs[1:])

    # per-slot K: max count over the 8 cores' tiles in that slot
    cnts_g = cnts.reshape(C, T, P)
    Ks = np.maximum(cnts_g.max(axis=(0, 2)), 1).astype(np.int64)   # [T]

    IDXC = int(8 * Ks.sum())
    WTC = int(Ks.sum())
    idx_dev = np.zeros((C, P, IDXC), np.int16)
    wt_dev = np.zeros((C, P, WTC), np.float32)
    nm = node_mask[:, 0].astype(np.float64)
    ioff = woff = 0
    ar = None
    for t in range(T):
        K = int(Ks[t])
        if ar is None or ar.shape[1] != K:
            ar = np.arange(K)[None, :]
        for cc in range(C):
            base = cc * NL + t * P
            cn = cnts[base:base + P]
            take = offs[base:base + P][:, None] + ar          # [P, K]
            valid = ar < cn[:, None]
            take_c = np.minimum(take, E - 1)
            nb = np.where(valid, src_sorted[take_c], 0)
            wl = np.where(valid, w_sorted[take_c], 0.0)
            wl = wl * nm[perm[base:base + P]][:, None]
            il = nb.T.reshape(-1)                             # i = g*128+p
            ch = il.reshape(8 * K, 16).T                      # [16, 8K]
            idx_dev[cc, :, ioff:ioff + 8 * K] = np.tile(ch, (8, 1)).astype(np.int16)
            wt_dev[cc, :, woff:woff + K] = wl.astype(np.float32)
        ioff += 8 * K
        woff += K
    # pad counts per (core, slot, partition) for the pad-subtract path
    pc_dev = np.zeros((C, 1, T * P), np.float32)
    for t in range(T):
        K = int(Ks[t])
        for cc in range(C):
            base = cc * NL + t * P
            pc_dev[cc, 0, t * P:(t + 1) * P] = K - cnts[base:base + P]
    allones = bool(np.all(edge_mask == 1.0) and np.all(node_mask == 1.0))
    return perm, Ks, idx_dev, wt_dev, IDXC, WTC, pc_dev, allones


def _build_program(Ks, IDXC, WTC, use_wt=True, sim=False):
    import concourse.bacc as bacc
    import concourse.bass as bass
    import concourse.mybir as mybir
    import concourse.tile as tile
    from concourse import library_config
    from concourse.masks import make_identity

    f32 = mybir.dt.float32
    i16 = mybir.dt.int16
    AF = mybir.ActivationFunctionType
    OP = mybir.AluOpType
    X = mybir.AxisListType.X

    nc = bacc.Bacc("TRN2", target_bir_lowering=False, debug=False,
                   num_devices=1 if sim else C)

    h_in = nc.dram_tensor("h_in", [P, T * D], f32, kind="ExternalInput")
    hT_in = nc.dram_tensor("hT_in", [D, T * P], f32, kind="ExternalInput")
    idx_in = nc.dram_tensor("idx_in", [P, IDXC], i16, kind="ExternalInput")
    wt_in = nc.dram_tensor("wt_in", [P, WTC], f32, kind="ExternalInput")
    w0t_in = nc.dram_tensor("w0t_in", [D, D], f32, kind="ExternalInput")
    w1t_in = nc.dram_tensor("w1t_in", [D, D], f32, kind="ExternalInput")
    wot_in = nc.dram_tensor("wot_in", [D, 16], f32, kind="ExternalInput")
    pc_in = nc.dram_tensor("pc_in", [1, T * P], f32, kind="ExternalInput")
    out_dram = nc.dram_tensor("out", [P, T * 16], f32, kind="ExternalOutput")
    xt_shard = nc.dram_tensor("xt_shard", [P, T * D], f32)
    xt_table = nc.dram_tensor("xt_table", [N, D], f32, addr_space="Shared")
    groups = [list(range(C))]

    ioffs = np.zeros(T + 1, dtype=np.int64)
    np.cumsum(8 * Ks, out=ioffs[1:])
    woffs = np.zeros(T + 1, dtype=np.int64)
    np.cumsum(Ks, out=woffs[1:])

    with tile.TileContext(nc) as tc:
        nc.gpsimd.load_library(library_config.mlp)
        import contextlib
        ctx = contextlib.ExitStack()
        with ctx:
            const = ctx.enter_context(tc.tile_pool(name="const", bufs=1))
            xtp = ctx.enter_context(tc.tile_pool(name="xtp", bufs=3))
            gp = ctx.enter_context(tc.tile_pool(name="gp", bufs=6))
            scp = ctx.enter_context(tc.tile_pool(name="scp", bufs=2))
            psp = ctx.enter_context(tc.tile_pool(name="psp", bufs=2, space="PSUM"))
            psmv = ctx.enter_context(tc.tile_pool(name="psmv", bufs=2, space="PSUM"))
            psc = ctx.enter_context(tc.tile_pool(name="psc", bufs=2, space="PSUM"))

            # ---- inputs ------------------------------------------------------
            x_sb = const.tile([P, T * D], f32)      # h (layer-0 input)
            nc.sync.dma_start(out=x_sb[:], in_=h_in[:])
            hT_sb = const.tile([D, T * P], f32)     # host-transposed h
            nc.sync.dma_start(out=hT_sb[:], in_=hT_in[:])
            w0t_sb = const.tile([D, D], f32)
            nc.sync.dma_start(out=w0t_sb[:], in_=w0t_in[:])
            w1t_sb = const.tile([D, D], f32)
            nc.sync.dma_start(out=w1t_sb[:], in_=w1t_in[:])
            wot_sb = const.tile([D, 16], f32)
            nc.sync.dma_start(out=wot_sb[:], in_=wot_in[:])
            idx_sb = const.tile([P, IDXC], i16)
            nc.sync.dma_start(out=idx_sb[:], in_=idx_in[:])
            if use_wt:
                wt_sb = const.tile([P, WTC], f32)
                nc.sync.dma_start(out=wt_sb[:], in_=wt_in[:])
            else:
                pc_sb = const.tile([1, T * P], f32)
                nc.sync.dma_start(out=pc_sb[:], in_=pc_in[:])

            ident = const.tile([P, P], f32)
            make_identity(nc, ident[:])

            mv_sb = const.tile([P, T * D], f32)     # raw W^T u, then messages
            agg_sb = const.tile([P, T * D], f32)    # aggregated tangent
            u_sb = const.tile([P, T * D], f32)      # relu'd tangent
            sq_sb = const.tile([P, T * D], f32)     # squares scratch
            out_sb = const.tile([P, T * 16], f32)
            nu2_all = const.tile([P, T], f32)       # |u|^2 for batched s3

            def ts(t, w=D):
                return slice(t * w, (t + 1) * w)

            def cs(lo, n, w=D):
                return slice(lo * w, (lo + n) * w)

            # ---- scalar-chain helpers (Rsqrt only => one act table) ---------
            # Key identities: artanh(min(tanh(min(x,15)), MAXN)) = min(x,
            # ARTMAXN), so logmap0(proj(expmap0(.))) collapses to the scale
            # min(1, ARTMAXN/|x|) = min(1, ARTMAXN*rsqrt(|x|^2)), and the full
            # matvec message scale to min(1, ART*rsqrt(|u|^2),
            # ART*rsqrt(|Wu|^2)).  Only the readout needs a real tanh (batched
            # once at the end).  ARTMAXN folds into Rsqrt's input scale.
            def rsq(dst, sq):
                """dst = ARTMAXN * rsqrt(max(sq, tiny)), in place allowed.
                sqrt(x/ART^2) = sqrt(x)/ART, then DVE reciprocal."""
                nc.vector.tensor_scalar_max(dst[:], sq[:], 1e-30)
                nc.scalar.activation(dst[:], dst[:], AF.Sqrt, scale=IART2)
                nc.vector.reciprocal(dst[:], dst[:])

            def rsq_min1(dst, sq):
                """dst = min(ARTMAXN * rsqrt(sq), 1)."""
                rsq(dst, sq)
                nc.vector.tensor_scalar_min(dst[:], dst[:], 1.0)

            def bcast(ap, w=D):
                """[P, W] -> [P, W, w] stride-0 broadcast view."""
                return bass.AP(ap.tensor, ap.offset, list(ap.ap) + [[0, w]])

            def sqreduce(dst, src_cols):
                """dst[P, w] = per-tile sum of squares already in sq_sb cols."""
                nc.vector.tensor_reduce(
                    dst[:], sq_sb[:, src_cols].rearrange(
                        "p (w d) -> p w d", d=D), axis=X, op=OP.add)

            def matvec(src, w_l, tlist):
                """Per tile: transpose src tile, matmul with w_l into PSUM,
                square into sq_sb, copy raw product into mv_sb."""
                for t in tlist:
                    xT_ps = psp.tile([D, P], f32, tag="xT")
                    nc.tensor.transpose(out=xT_ps[:], in_=src[:, ts(t)],
                                        identity=ident[:])
                    xT = xtp.tile([D, P], f32, tag="xT_sb")
                    nc.scalar.activation(xT[:], xT_ps[:], AF.Copy)
                    mv_ps = psmv.tile([P, D], f32, tag="mv")
                    nc.tensor.matmul(out=mv_ps[:], lhsT=xT[:], rhs=w_l[:],
                                     start=True, stop=True)
                    nc.scalar.activation(sq_sb[:, ts(t)], mv_ps[:], AF.Square)
                    nc.scalar.activation(mv_sb[:, ts(t)], mv_ps[:], AF.Copy)

            def msg_scale(r1, cols, w, tag):
                """Apply scl = min(r1, ART*rsqrt(|mv|^2)) to mv_sb[:, cols];
                r1 = min(ART*rsqrt(|x|^2), 1) from the layer input, mv squares
                already in sq_sb[:, cols]."""
                m2 = scp.tile([P, w], f32, tag=f"m2{tag}")
                sqreduce(m2, cols)
                rsq(m2, m2)
                nc.vector.tensor_tensor(m2[:], m2[:], r1[:], op=OP.min)
                mv3 = mv_sb[:, cols].rearrange("p (w d) -> p w d", d=D)
                nc.vector.tensor_tensor(mv3, mv3, bcast(m2[:]), op=OP.mult)

            def allgather():
                tc.strict_bb_all_engine_barrier()
                if sim:
                    nc.sync.dma_start(
                        out=xt_table[0:NL, :].rearrange("(p x) d -> p x d", p=P),
                        in_=xt_shard[:].rearrange("p (x d) -> p x d", d=D))
                else:
                    nc.gpsimd.collective_compute(
                        "AllGather", mybir.AluOpType.bypass,
                        replica_groups=groups,
                        ins=[xt_shard[:, :]], outs=[xt_table[:, :]])
                tc.strict_bb_all_engine_barrier()

            # ---- head: layer-0 messages straight from h ---------------------
            # x0 = proj(expmap0(h)); msg = logmap0(proj(mobius_matvec(W0,x0)))
            #     = scl * (W^T h) with scl = min(1, ART*rsqrt(|h|^2),
            # ART*rsqrt(|Wh|^2)); host supplies hT so no transposes needed.
            # Grouped by 8 tiles so scale/publish of group g overlaps the
            # matmuls of group g+1.
            for g0 in range(0, T, 8):
                gcols = cs(g0, 8)
                nc.scalar.activation(sq_sb[:, gcols], x_sb[:, gcols],
                                     AF.Square)
                n2g = scp.tile([P, 8], f32, tag="n2h")
                sqreduce(n2g, gcols)
                rsq_min1(n2g, n2g)
                for t in range(g0, g0 + 8):
                    mv_ps = psmv.tile([P, D], f32, tag="mv")
                    nc.tensor.matmul(out=mv_ps[:], lhsT=hT_sb[:, ts(t, P)],
                                     rhs=w0t_sb[:], start=True, stop=True)
                    nc.scalar.activation(sq_sb[:, ts(t)], mv_ps[:], AF.Square)
                    nc.scalar.activation(mv_sb[:, ts(t)], mv_ps[:], AF.Copy)
                msg_scale(n2g, gcols, 8, "h")
                nc.sync.dma_start(out=xt_shard[:, gcols],
                                  in_=mv_sb[:, gcols])
            allgather()

            # ---- layers: gather stream with chunked trailing compute --------
            for layer in range(2):
                if not use_wt:
                    row0_sb = scp.tile([1, D], f32, tag="row0")
                    nc.sync.dma_start(out=row0_sb[:], in_=xt_table[0:1, :])
                for lo, nch in CHUNKS:
                    chunk = list(range(lo, lo + nch))
                    for t in chunk:
                        K = int(Ks[t])
                        io, wo = int(ioffs[t]), int(woffs[t])
                        g = gp.tile([P, K * D], f32, tag="G")
                        g3 = g[:].rearrange("p (k d) -> p k d", d=D)
                        nc.gpsimd.dma_gather(
                            g3, xt_table[:, :], idx_sb[:, io:io + 8 * K],
                            num_idxs=P * K, num_idxs_reg=P * K, elem_size=D,
                            single_packet=False)
                        if use_wt:
                            wt_ap = wt_sb[:, wo:wo + K]
                            nc.vector.tensor_tensor(g3, g3, bcast(wt_ap),
                                                    op=OP.mult)
                        nc.vector.tensor_reduce(
                            agg_sb[:, ts(t)],
                            g[:].rearrange("p (k d) -> p d k", d=D),
                            axis=X, op=OP.add)
                        if not use_wt:
                            corr_ps = psc.tile([P, D], f32, tag="corr")
                            nc.tensor.matmul(
                                out=corr_ps[:],
                                lhsT=pc_sb[0:1, t * P:(t + 1) * P],
                                rhs=row0_sb[0:1, :], start=True, stop=True)
                            nc.vector.tensor_tensor(agg_sb[:, ts(t)],
                                                    agg_sb[:, ts(t)],
                                                    corr_ps[:],
                                                    op=OP.subtract)
                    # chunk chain: u = relu(s2*agg),
                    # s2 = min(ART*rsqrt(|agg|^2), 1)  [collapsed identity]
                    ccols = cs(lo, nch)
                    nc.scalar.activation(sq_sb[:, ccols], agg_sb[:, ccols],
                                         AF.Square)
                    s2 = scp.tile([P, nch], f32, tag="s2c")
                    sqreduce(s2, ccols)
                    rsq_min1(s2, s2)
                    u3 = u_sb[:, ccols].rearrange("p (w d) -> p w d", d=D)
                    nc.vector.tensor_tensor(
                        u3, agg_sb[:, ccols].rearrange("p (w d) -> p w d", d=D),
                        bcast(s2[:]), op=OP.mult)
                    nc.scalar.activation(u_sb[:, ccols], u_sb[:, ccols],
                                         AF.Relu)
                    nc.scalar.activation(sq_sb[:, ccols], u_sb[:, ccols],
                                         AF.Square)
                    if layer == 0:
                        r1 = scp.tile([P, nch], f32, tag="r1c")
                        sqreduce(r1, ccols)
                        rsq_min1(r1, r1)
                        # next layer's raw matvec + message scale, publish
                        matvec(u_sb, w1t_sb, chunk)
                        msg_scale(r1, ccols, nch, "c")
                        nc.sync.dma_start(out=xt_shard[:, ccols],
                                          in_=mv_sb[:, ccols])
                    else:
                        # readout: raw u @ W_out^T now; the per-node scale
                        # s3 = min(tanh(|u|),MAXN)/|u| is batched at the end
                        # (the only real tanh -> one table switch total)
                        nc.vector.tensor_reduce(
                            nu2_all[:, lo:lo + nch],
                            sq_sb[:, ccols].rearrange("p (w d) -> p w d", d=D),
                            axis=X, op=OP.add)
                        for t in chunk:
                            xT_ps = psp.tile([D, P], f32, tag="xT")
                            nc.tensor.transpose(out=xT_ps[:],
                                                in_=u_sb[:, ts(t)],
                                                identity=ident[:])
                            xT = xtp.tile([D, P], f32, tag="xT_sb")
                            nc.scalar.activation(xT[:], xT_ps[:], AF.Copy)
                            o_ps = psmv.tile([P, 16], f32, tag="mv")
                            nc.tensor.matmul(out=o_ps[:], lhsT=xT[:],
                                             rhs=wot_sb[:], start=True,
                                             stop=True)
                            nc.scalar.activation(out_sb[:, ts(t, 16)], o_ps[:],
                                                 AF.Copy)
                if layer == 0:
                    allgather()

            # batched readout scale: s3 = min(tanh(min(nu,15)), MAXN)/nu with
            # tanh = 1-2/(1+e^2x); rr = rsqrt(nu2), nu = nu2*rr, s3 = a3*rr
            rr = scp.tile([P, T], f32, tag="rr")
            nc.vector.tensor_scalar_max(rr[:], nu2_all[:], 1e-30)
            nc.scalar.activation(rr[:], rr[:], AF.Sqrt)
            nc.vector.reciprocal(rr[:], rr[:])
            a3 = scp.tile([P, T], f32, tag="a3")
            nc.vector.tensor_tensor(a3[:], nu2_all[:], rr[:], op=OP.mult)
            nc.vector.tensor_scalar_min(a3[:], a3[:], MAX_TANH)
            nc.scalar.activation(a3[:], a3[:], AF.Exp, scale=2.0)
            nc.scalar.activation(a3[:], a3[:], AF.Copy, bias=1.0)
            nc.vector.reciprocal(a3[:], a3[:])
            nc.scalar.activation(a3[:], a3[:], AF.Copy, scale=-2.0, bias=1.0)
            nc.vector.tensor_scalar_min(a3[:], a3[:], MAXN)
            nc.vector.tensor_tensor(a3[:], a3[:], rr[:], op=OP.mult)
            out3 = out_sb[:].rearrange("p (t o) -> p t o", o=16)
            nc.vector.tensor_tensor(out3, out3, bcast(a3[:], w=16),
                                    op=OP.mult)
            nc.sync.dma_start(out=out_dram[:], in_=out_sb[:])
    nc.compile()
    return nc


def kernel(h, distances, rows, cols, node_mask, edge_mask,
           W0, b0, W1, b1, W_out, b_out, _trace=False):
    from concourse.bass_utils import run_bass_kernel_spmd

    h = np.asarray(h, dtype=np.float32)
    rows = np.asarray(rows).astype(np.int64)
    cols = np.asarray(cols).astype(np.int64)
    node_mask = np.asarray(node_mask, dtype=np.float32)
    edge_mask = np.asarray(edge_mask, dtype=np.float32)
    assert not np.any(np.asarray(b0)) and not np.any(np.asarray(b1)) and \
        not np.any(np.asarray(b_out)), "nonzero biases unsupported"

    perm, Ks, idx_dev, wt_dev, IDXC, WTC, pc_dev, allones = _build_tables(
        rows, cols, edge_mask, node_mask)

    hp = h[perm].reshape(C, T, P, D).transpose(0, 2, 1, 3).reshape(C, P, T * D)
    # hT[c][d, t*P+p] = h[node (c,t,p), d] for transpose-free head matmuls
    hT = h[perm].reshape(C, T, P, D).transpose(0, 3, 1, 2).reshape(C, D, T * P)
    w0t = np.ascontiguousarray(np.asarray(W0, np.float32).T)
    w1t = np.ascontiguousarray(np.asarray(W1, np.float32).T)
    wot = np.ascontiguousarray(np.asarray(W_out, np.float32).T)

    nc = _build_program(Ks, IDXC, WTC, use_wt=not allones)
    in_maps = [{
        "h_in": np.ascontiguousarray(hp[c]),
        "hT_in": np.ascontiguousarray(hT[c]),
        "idx_in": idx_dev[c],
        "wt_in": wt_dev[c],
        "w0t_in": w0t, "w1t_in": w1t, "wot_in": wot,
        "pc_in": pc_dev[c],
    } for c in range(C)]
    res = run_bass_kernel_spmd(nc, in_maps, list(range(C)), trace=_trace)
    od = np.stack([res.results[c]["out"] for c in range(C)])
    od = od.reshape(C, P, T, 16).transpose(0, 2, 1, 3).reshape(N, 16)
    out = np.empty((N, 16), np.float32)
    out[perm] = od
    if _trace:
        return out, res
    return out


# revision 41
# speedup vs baseline: 1.3567x; 1.0304x over previous
"""HGCN decoder on 8 trn2 NeuronCores.

Strategy: nodes are sorted by in-degree, grouped into 128-node tiles, and the
tiles are dealt round-robin across the 8 cores (graph-parallel by destination
node).  Per layer the kernel runs a continuous gather-DMA stream (the cost
floor) and hides everything else beneath it:
  - tiles are gathered smallest-degree-first in chunks of 8; each chunk's
    weighted reduce, hyperbolic chain math, relu, the NEXT layer's matvec
    (transpose + matmul on PE) and the final readout all ride under the
    remaining gather stream,
  - all per-node scalar chains use only {Square, Copy, Exp, Ln, Relu} so the
    whole program needs a single activation-table load (tanh/sqrt/artanh are
    rewritten in exp/ln form),
  - manifold scale factors cancel analytically through the matvec: the
    published tangent message is (artanh(at)/|Wu|) * (W^T u) with
    at = min(tanh(|Wu|/|u| * artanh(a)), 1-eps), so no explicit manifold
    tensor is ever materialized.
All graph preprocessing (permutation, padded neighbor tables, pad-count
correction) happens host-side in numpy; the device only sees dense tables.
"""

import numpy as np

N = 32768
E = 1015808
D = 64
C = 8          # cores
NL = N // C    # 4096 nodes per core
P = 128        # partitions / tile
T = NL // P    # 32 tiles per core
MAXN = 1.0 - 4e-3   # PROJ_EPS boundary for c=1
ARTMAXN = 3.106303047875759   # artanh(MAXN)
IART2 = 1.0 / (ARTMAXN * ARTMAXN)
EPS = 1e-15
MAX_TANH = 15.0
# contiguous slot chunks in stream order: a small warmup chunk, the big-K
# chunks in the middle, and progressively tinier chunks at the end so the
# trailing compute exposed at layer boundaries shrinks to a single tile
CHUNKS = ((20, 4), (0, 8), (8, 8), (16, 4), (24, 2), (26, 2), (28, 2),
          (30, 1), (31, 1))


def _build_tables(rows, cols, edge_mask, node_mask):
    """Permute nodes by degree, deal tiles round-robin to cores, and build the
    per-core padded gather tables (int16 indices wrapped the way
    InstDMAGatherAnt wants them) plus matching weight tables."""
    deg = np.bincount(rows, minlength=N)
    order = np.argsort(-deg, kind="stable")
    # global tile j -> core j%C, slot j//C ; permuted position of its p-th node
    perm = np.empty(N, dtype=np.int64)
    j = np.arange(N) // P                     # global tile of sorted rank r
    c = j % C
    t = j // C
    p = np.arange(N) % P
    perm[c * NL + t * P + p] = order          # perm[g] = original node id
    pos = np.empty(N, dtype=np.int64)
    pos[perm] = np.arange(N)

    # gather-table row id for permuted position g=(c,t,p):
    #   AllGather concatenates per-core [P, T*D] blocks, so
    #   row_id = c*NL + p*T + t
    gg = np.arange(N)
    gc, gr = gg // NL, gg % NL
    gt, gp_ = gr // P, gr % P
    rowid = gc * NL + gp_ * T + gt            # [g] -> table row
    dstpos = pos[rows]
    eorder = np.argsort(dstpos, kind="stable")
    src_sorted = rowid[pos[cols[eorder]]]     # gather table rows, 0..N-1
    w_sorted = edge_mask[eorder, 0].astype(np.float64)
    cnts = np.bincount(dstpos, minlength=N)
    offs = np.zeros(N + 1, dtype=np.int64)
    np.cumsum(cnts, out=offs[1:])

    # per-slot K: max count over the 8 cores' tiles in that slot
    cnts_g = cnts.reshape(C, T, P)
    Ks = np.maximum(cnts_g.max(axis=(0, 2)), 1).astype(np.int64)   # [T]

    IDXC = int(8 * Ks.sum())
    WTC = int(Ks.sum())
    idx_dev = np.zeros((C, P, IDXC), np.int16)
    wt_dev = np.zeros((C, P, WTC), np.float32)
    nm = node_mask[:, 0].astype(np.float64)
    ioff = woff = 0
    ar = None
    for t in range(T):
        K = int(Ks[t])
        if ar is None or ar.shape[1] != K:
            ar = np.arange(K)[None, :]
        for cc in range(C):
            base = cc * NL + t * P
            cn = cnts[base:base + P]
            take = offs[base:base + P][:, None] + ar          # [P, K]
            valid = ar < cn[:, None]
            take_c = np.minimum(take, E - 1)
            nb = np.where(valid, src_sorted[take_c], 0)
            wl = np.where(valid, w_sorted[take_c], 0.0)
            wl = wl * nm[perm[base:base + P]][:, None]
            il = nb.T.reshape(-1)                             # i = g*128+p
            ch = il.reshape(8 * K, 16).T                      # [16, 8K]
            idx_dev[cc, :, ioff:ioff + 8 * K] = np.tile(ch, (8, 1)).astype(np.int16)
            wt_dev[cc, :, woff:woff + K] = wl.astype(np.float32)
        ioff += 8 * K
        woff += K
    # transposed pad counts per chunk for the block-diagonal pad-subtract:
    # pcT[c][w, ci*P + p] = #pad entries of tile (lo_ci + w) partition p
    NCH = len(CHUNKS)
    pc_dev = np.zeros((C, 8, NCH * P), np.float32)
    for ci, (lo, n) in enumerate(CHUNKS):
        for w in range(n):
            t = lo + w
            K = int(Ks[t])
            for cc in range(C):
                base = cc * NL + t * P
                pc_dev[cc, w, ci * P:(ci + 1) * P] = K - cnts[base:base + P]
    allones = bool(np.all(edge_mask == 1.0) and np.all(node_mask == 1.0))
    return perm, Ks, idx_dev, wt_dev, IDXC, WTC, pc_dev, allones


def _build_program(Ks, IDXC, WTC, use_wt=True, sim=False):
    import concourse.bacc as bacc
    import concourse.bass as bass
    import concourse.mybir as mybir
    import concourse.tile as tile
    from concourse import library_config
    from concourse.masks import make_identity

    f32 = mybir.dt.float32
    i16 = mybir.dt.int16
    AF = mybir.ActivationFunctionType
    OP = mybir.AluOpType
    X = mybir.AxisListType.X

    nc = bacc.Bacc("TRN2", target_bir_lowering=False, debug=False,
                   num_devices=1 if sim else C)

    h_in = nc.dram_tensor("h_in", [P, T * D], f32, kind="ExternalInput")
    hT_in = nc.dram_tensor("hT_in", [D, T * P], f32, kind="ExternalInput")
    idx_in = nc.dram_tensor("idx_in", [P, IDXC], i16, kind="ExternalInput")
    wt_in = nc.dram_tensor("wt_in", [P, WTC], f32, kind="ExternalInput")
    w0t_in = nc.dram_tensor("w0t_in", [D, D], f32, kind="ExternalInput")
    w1t_in = nc.dram_tensor("w1t_in", [D, D], f32, kind="ExternalInput")
    wot_in = nc.dram_tensor("wot_in", [D, 16], f32, kind="ExternalInput")
    pc_in = nc.dram_tensor("pc_in", [8, len(CHUNKS) * P], f32,
                           kind="ExternalInput")
    out_dram = nc.dram_tensor("out", [P, T * 16], f32, kind="ExternalOutput")
    xt_shard = nc.dram_tensor("xt_shard", [P, T * D], f32)
    xt_table = nc.dram_tensor("xt_table", [N, D], f32, addr_space="Shared")
    groups = [list(range(C))]

    ioffs = np.zeros(T + 1, dtype=np.int64)
    np.cumsum(8 * Ks, out=ioffs[1:])
    woffs = np.zeros(T + 1, dtype=np.int64)
    np.cumsum(Ks, out=woffs[1:])

    with tile.TileContext(nc) as tc:
        nc.gpsimd.load_library(library_config.mlp)
        import contextlib
        ctx = contextlib.ExitStack()
        with ctx:
            const = ctx.enter_context(tc.tile_pool(name="const", bufs=1))
            xtp = ctx.enter_context(tc.tile_pool(name="xtp", bufs=3))
            gp = ctx.enter_context(tc.tile_pool(name="gp", bufs=6))
            scp = ctx.enter_context(tc.tile_pool(name="scp", bufs=2))
            psp = ctx.enter_context(tc.tile_pool(name="psp", bufs=2, space="PSUM"))
            psmv = ctx.enter_context(tc.tile_pool(name="psmv", bufs=4, space="PSUM"))
            psc = ctx.enter_context(tc.tile_pool(name="psc", bufs=2, space="PSUM"))

            # ---- inputs ------------------------------------------------------
            x_sb = const.tile([P, T * D], f32)      # h (layer-0 input)
            nc.sync.dma_start(out=x_sb[:], in_=h_in[:])
            hT_sb = const.tile([D, T * P], f32)     # host-transposed h
            nc.sync.dma_start(out=hT_sb[:], in_=hT_in[:])
            w0t_sb = const.tile([D, D], f32)
            nc.sync.dma_start(out=w0t_sb[:], in_=w0t_in[:])
            w1t_sb = const.tile([D, D], f32)
            nc.sync.dma_start(out=w1t_sb[:], in_=w1t_in[:])
            wot_sb = const.tile([D, 16], f32)
            nc.sync.dma_start(out=wot_sb[:], in_=wot_in[:])
            idx_sb = const.tile([P, IDXC], i16)
            nc.sync.dma_start(out=idx_sb[:], in_=idx_in[:])
            if use_wt:
                wt_sb = const.tile([P, WTC], f32)
                nc.sync.dma_start(out=wt_sb[:], in_=wt_in[:])
            else:
                pc_sb = const.tile([8, len(CHUNKS) * P], f32)
                nc.sync.dma_start(out=pc_sb[:], in_=pc_in[:])
                # block-diagonal [8, 8*D]: row w holds row0 at cols w*D..;
                # the diagonal blocks are re-DMA'd from each layer's table
                br0_sb = const.tile([8, 8 * D], f32)
                nc.vector.memset(br0_sb[:], 0.0)

            ident = const.tile([P, P], f32)
            make_identity(nc, ident[:])

            mv_sb = const.tile([P, T * D], f32)     # raw W^T u, then messages
            agg_sb = const.tile([P, T * D], f32)    # aggregated tangent
            u_sb = const.tile([P, T * D], f32)      # relu'd tangent
            sq_sb = const.tile([P, T * D], f32)     # squares scratch
            out_sb = const.tile([P, T * 16], f32)
            nu2_all = const.tile([P, T], f32)       # |u|^2 for batched s3

            def ts(t, w=D):
                return slice(t * w, (t + 1) * w)

            def cs(lo, n, w=D):
                return slice(lo * w, (lo + n) * w)

            # ---- scalar-chain helpers (Rsqrt only => one act table) ---------
            # Key identities: artanh(min(tanh(min(x,15)), MAXN)) = min(x,
            # ARTMAXN), so logmap0(proj(expmap0(.))) collapses to the scale
            # min(1, ARTMAXN/|x|) = min(1, ARTMAXN*rsqrt(|x|^2)), and the full
            # matvec message scale to min(1, ART*rsqrt(|u|^2),
            # ART*rsqrt(|Wu|^2)).  Only the readout needs a real tanh (batched
            # once at the end).  ARTMAXN folds into Rsqrt's input scale.
            def rsq(dst, sq):
                """dst = ARTMAXN * rsqrt(max(sq, tiny)), in place allowed.
                sqrt(x/ART^2) = sqrt(x)/ART, then DVE reciprocal."""
                nc.vector.tensor_scalar_max(dst[:], sq[:], 1e-30)
                nc.scalar.activation(dst[:], dst[:], AF.Sqrt, scale=IART2)
                nc.vector.reciprocal(dst[:], dst[:])

            def rsq_min1(dst, sq):
                """dst = min(ARTMAXN * rsqrt(sq), 1)."""
                rsq(dst, sq)
                nc.vector.tensor_scalar_min(dst[:], dst[:], 1.0)

            def bcast(ap, w=D):
                """[P, W] -> [P, W, w] stride-0 broadcast view."""
                return bass.AP(ap.tensor, ap.offset, list(ap.ap) + [[0, w]])

            def sqreduce(dst, src_cols):
                """dst[P, w] = per-tile sum of squares already in sq_sb cols."""
                nc.vector.tensor_reduce(
                    dst[:], sq_sb[:, src_cols].rearrange(
                        "p (w d) -> p w d", d=D), axis=X, op=OP.add)

            def matvec(src, w_l, tlist):
                """Per tile: transpose src tile, matmul with w_l into PSUM,
                square into sq_sb (Act), copy raw product to mv_sb (DVE)."""
                for t in tlist:
                    xT_ps = psp.tile([D, P], f32, tag="xT")
                    nc.tensor.transpose(out=xT_ps[:], in_=src[:, ts(t)],
                                        identity=ident[:])
                    xT = xtp.tile([D, P], f32, tag="xT_sb")
                    nc.scalar.activation(xT[:], xT_ps[:], AF.Copy)
                    mv_ps = psmv.tile([P, D], f32, tag="mv")
                    nc.tensor.matmul(out=mv_ps[:], lhsT=xT[:], rhs=w_l[:],
                                     start=True, stop=True)
                    nc.scalar.activation(sq_sb[:, ts(t)], mv_ps[:], AF.Square)
                    nc.vector.tensor_copy(mv_sb[:, ts(t)], mv_ps[:])

            def msg_scale(r1_ap, cols, w, tag):
                """scl = min(r1, ART*rsqrt(|mv|^2)) applied to mv_sb[:, cols];
                r1_ap = min(ART*rsqrt(|x|^2), 1) from the layer input, mv
                squares already in sq_sb[:, cols]."""
                m2 = scp.tile([P, w], f32, tag=f"m2{tag}")
                sqreduce(m2, cols)
                rsq(m2, m2)
                nc.vector.tensor_tensor(m2[:], m2[:], r1_ap, op=OP.min)
                mv3 = mv_sb[:, cols].rearrange("p (w d) -> p w d", d=D)
                nc.vector.tensor_tensor(mv3, mv3, bcast(m2[:]), op=OP.mult)

            def allgather():
                tc.strict_bb_all_engine_barrier()
                if sim:
                    nc.sync.dma_start(
                        out=xt_table[0:NL, :].rearrange("(p x) d -> p x d", p=P),
                        in_=xt_shard[:].rearrange("p (x d) -> p x d", d=D))
                else:
                    nc.gpsimd.collective_compute(
                        "AllGather", mybir.AluOpType.bypass,
                        replica_groups=groups,
                        ins=[xt_shard[:, :]], outs=[xt_table[:, :]])
                tc.strict_bb_all_engine_barrier()

            # ---- head: layer-0 messages straight from h ---------------------
            # x0 = proj(expmap0(h)); msg = logmap0(proj(mobius_matvec(W0,x0)))
            #     = scl * (W^T h) with scl = min(1, ART*rsqrt(|h|^2),
            # ART*rsqrt(|Wh|^2)); host supplies hT so no transposes needed.
            # Grouped by 8 tiles so scale/publish of group g overlaps the
            # matmuls of group g+1; mv squares on Act, copies on DVE.
            for g0 in range(0, T, 8):
                gcols = cs(g0, 8)
                nc.scalar.activation(sq_sb[:, gcols], x_sb[:, gcols],
                                     AF.Square)
                n2g = scp.tile([P, 8], f32, tag="n2h")
                sqreduce(n2g, gcols)
                rsq_min1(n2g, n2g)
                for t in range(g0, g0 + 8):
                    mv_ps = psmv.tile([P, D], f32, tag="mv")
                    nc.tensor.matmul(out=mv_ps[:], lhsT=hT_sb[:, ts(t, P)],
                                     rhs=w0t_sb[:], start=True, stop=True)
                    nc.scalar.activation(sq_sb[:, ts(t)], mv_ps[:], AF.Square)
                    nc.vector.tensor_copy(mv_sb[:, ts(t)], mv_ps[:])
                msg_scale(n2g[:], gcols, 8, "h")
                nc.sync.dma_start(out=xt_shard[:, gcols],
                                  in_=mv_sb[:, gcols])
            allgather()

            # ---- layers: gather stream with chunked trailing compute --------
            for layer in range(2):
                if not use_wt:
                    for w in range(8):
                        nc.sync.dma_start(
                            out=br0_sb[w:w + 1, w * D:(w + 1) * D],
                            in_=xt_table[0:1, :])
                for ci, (lo, nch) in enumerate(CHUNKS):
                    chunk = list(range(lo, lo + nch))
                    if not use_wt:
                        corr_ps = psc.tile([P, nch * D], f32, tag="corr")
                        nc.tensor.matmul(
                            out=corr_ps[:],
                            lhsT=pc_sb[:, ci * P:(ci + 1) * P],
                            rhs=br0_sb[:, 0:nch * D], start=True, stop=True)
                    for t in chunk:
                        K = int(Ks[t])
                        io, wo = int(ioffs[t]), int(woffs[t])
                        g = gp.tile([P, K * D], f32, tag="G")
                        g3 = g[:].rearrange("p (k d) -> p k d", d=D)
                        nc.gpsimd.dma_gather(
                            g3, xt_table[:, :], idx_sb[:, io:io + 8 * K],
                            num_idxs=P * K, num_idxs_reg=P * K, elem_size=D,
                            single_packet=False)
                        if use_wt:
                            wt_ap = wt_sb[:, wo:wo + K]
                            nc.vector.tensor_tensor(g3, g3, bcast(wt_ap),
                                                    op=OP.mult)
                        nc.vector.tensor_reduce(
                            agg_sb[:, ts(t)],
                            g[:].rearrange("p (k d) -> p d k", d=D),
                            axis=X, op=OP.add)
                    if not use_wt:
                        nc.vector.tensor_tensor(
                            agg_sb[:, cs(lo, nch)], agg_sb[:, cs(lo, nch)],
                            corr_ps[:], op=OP.subtract)
                    # chunk chain: u = relu(s2*agg),
                    # s2 = min(ART*rsqrt(|agg|^2), 1)  [collapsed identity]
                    ccols = cs(lo, nch)
                    nc.scalar.activation(sq_sb[:, ccols], agg_sb[:, ccols],
                                         AF.Square)
                    s2 = scp.tile([P, nch], f32, tag="s2c")
                    sqreduce(s2, ccols)
                    rsq_min1(s2, s2)
                    u3 = u_sb[:, ccols].rearrange("p (w d) -> p w d", d=D)
                    nc.vector.tensor_tensor(
                        u3, agg_sb[:, ccols].rearrange("p (w d) -> p w d", d=D),
                        bcast(s2[:]), op=OP.mult)
                    nc.scalar.activation(u_sb[:, ccols], u_sb[:, ccols],
                                         AF.Relu)
                    nc.scalar.activation(sq_sb[:, ccols], u_sb[:, ccols],
                                         AF.Square)
                    if layer == 0:
                        r1 = scp.tile([P, nch], f32, tag="r1c")
                        sqreduce(r1, ccols)
                        rsq_min1(r1, r1)
                        # next layer's raw matvec + message scale, publish
                        matvec(u_sb, w1t_sb, chunk)
                        msg_scale(r1[:], ccols, nch, "c")
                        nc.sync.dma_start(out=xt_shard[:, ccols],
                                          in_=mv_sb[:, ccols])
                    else:
                        # readout: raw u @ W_out^T now; the per-node scale
                        # s3 = min(tanh(|u|),MAXN)/|u| is batched at the end
                        # (the only real tanh -> one table switch total)
                        nc.vector.tensor_reduce(
                            nu2_all[:, lo:lo + nch],
                            sq_sb[:, ccols].rearrange("p (w d) -> p w d", d=D),
                            axis=X, op=OP.add)
                        for t in chunk:
                            xT_ps = psp.tile([D, P], f32, tag="xT")
                            nc.tensor.transpose(out=xT_ps[:],
                                                in_=u_sb[:, ts(t)],
                                                identity=ident[:])
                            xT = xtp.tile([D, P], f32, tag="xT_sb")
                            nc.scalar.activation(xT[:], xT_ps[:], AF.Copy)
                            o_ps = psmv.tile([P, 16], f32, tag="mv")
                            nc.tensor.matmul(out=o_ps[:], lhsT=xT[:],
                                             rhs=wot_sb[:], start=True,
                                             stop=True)
                            nc.scalar.activation(out_sb[:, ts(t, 16)], o_ps[:],
                                                 AF.Copy)
                if layer == 0:
                    allgather()

            # batched readout scale: s3 = min(tanh(min(nu,15)), MAXN)/nu with
            # tanh = 1-2/(1+e^2x); rr = rsqrt(nu2), nu = nu2*rr, s3 = a3*rr
            rr = scp.tile([P, T], f32, tag="rr")
            nc.vector.tensor_scalar_max(rr[:], nu2_all[:], 1e-30)
            nc.scalar.activation(rr[:], rr[:], AF.Sqrt)
            nc.vector.reciprocal(rr[:], rr[:])
            a3 = scp.tile([P, T], f32, tag="a3")
            nc.vector.tensor_tensor(a3[:], nu2_all[:], rr[:], op=OP.mult)
            nc.vector.tensor_scalar_min(a3[:], a3[:], MAX_TANH)
            nc.scalar.activation(a3[:], a3[:], AF.Exp, scale=2.0)
            nc.scalar.activation(a3[:], a3[:], AF.Copy, bias=1.0)
            nc.vector.reciprocal(a3[:], a3[:])
            nc.scalar.activation(a3[:], a3[:], AF.Copy, scale=-2.0, bias=1.0)
            nc.vector.tensor_scalar_min(a3[:], a3[:], MAXN)
            nc.vector.tensor_tensor(a3[:], a3[:], rr[:], op=OP.mult)
            out3 = out_sb[:].rearrange("p (t o) -> p t o", o=16)
            nc.vector.tensor_tensor(out3, out3, bcast(a3[:], w=16),
                                    op=OP.mult)
            nc.sync.dma_start(out=out_dram[:], in_=out_sb[:])
    nc.compile()
    return nc


def kernel(h, distances, rows, cols, node_mask, edge_mask,
           W0, b0, W1, b1, W_out, b_out, _trace=False):
    from concourse.bass_utils import run_bass_kernel_spmd

    h = np.asarray(h, dtype=np.float32)
    rows = np.asarray(rows).astype(np.int64)
    cols = np.asarray(cols).astype(np.int64)
    node_mask = np.asarray(node_mask, dtype=np.float32)
    edge_mask = np.asarray(edge_mask, dtype=np.float32)
    assert not np.any(np.asarray(b0)) and not np.any(np.asarray(b1)) and \
        not np.any(np.asarray(b_out)), "nonzero biases unsupported"

    perm, Ks, idx_dev, wt_dev, IDXC, WTC, pc_dev, allones = _build_tables(
        rows, cols, edge_mask, node_mask)

    hp = h[perm].reshape(C, T, P, D).transpose(0, 2, 1, 3).reshape(C, P, T * D)
    # hT[c][d, t*P+p] = h[node (c,t,p), d] for transpose-free head matmuls
    hT = h[perm].reshape(C, T, P, D).transpose(0, 3, 1, 2).reshape(C, D, T * P)
    w0t = np.ascontiguousarray(np.asarray(W0, np.float32).T)
    w1t = np.ascontiguousarray(np.asarray(W1, np.float32).T)
    wot = np.ascontiguousarray(np.asarray(W_out, np.float32).T)

    nc = _build_program(Ks, IDXC, WTC, use_wt=not allones)
    in_maps = [{
        "h_in": np.ascontiguousarray(hp[c]),
        "hT_in": np.ascontiguousarray(hT[c]),
        "idx_in": idx_dev[c],
        "wt_in": wt_dev[c],
        "w0t_in": w0t, "w1t_in": w1t, "wot_in": wot,
        "pc_in": pc_dev[c],
    } for c in range(C)]
    res = run_bass_kernel_spmd(nc, in_maps, list(range(C)), trace=_trace)
    od = np.stack([res.results[c]["out"] for c in range(C)])
    od = od.reshape(C, P, T, 16).transpose(0, 2, 1, 3).reshape(N, 16)
    out = np.empty((N, 16), np.float32)
    out[perm] = od
    if _trace:
        return out, res
    return out


# revision 57
# speedup vs baseline: 1.3606x; 1.0029x over previous
"""HGCN decoder on 8 trn2 NeuronCores.

Strategy: nodes are sorted by in-degree, grouped into 128-node tiles, and the
tiles are dealt round-robin across the 8 cores (graph-parallel by destination
node).  Per layer the kernel runs a continuous gather-DMA stream (the cost
floor) and hides everything else beneath it:
  - tiles are gathered smallest-degree-first in chunks of 8; each chunk's
    weighted reduce, hyperbolic chain math, relu, the NEXT layer's matvec
    (transpose + matmul on PE) and the final readout all ride under the
    remaining gather stream,
  - all per-node scalar chains use only {Square, Copy, Exp, Ln, Relu} so the
    whole program needs a single activation-table load (tanh/sqrt/artanh are
    rewritten in exp/ln form),
  - manifold scale factors cancel analytically through the matvec: the
    published tangent message is (artanh(at)/|Wu|) * (W^T u) with
    at = min(tanh(|Wu|/|u| * artanh(a)), 1-eps), so no explicit manifold
    tensor is ever materialized.
All graph preprocessing (permutation, padded neighbor tables, pad-count
correction) happens host-side in numpy; the device only sees dense tables.
"""

import numpy as np

N = 32768
E = 1015808
D = 64
C = 8          # cores
NL = N // C    # 4096 nodes per core
P = 128        # partitions / tile
T = NL // P    # 32 tiles per core
MAXN = 1.0 - 4e-3   # PROJ_EPS boundary for c=1
ARTMAXN = 3.106303047875759   # artanh(MAXN)
IART2 = 1.0 / (ARTMAXN * ARTMAXN)
EPS = 1e-15
MAX_TANH = 15.0
# contiguous slot chunks in stream order: a small warmup chunk, the big-K
# chunks in the middle, and progressively tinier chunks at the end so the
# trailing compute exposed at layer boundaries shrinks to a single tile
CHUNKS = ((20, 4), (0, 8), (8, 8), (16, 4), (24, 2), (26, 2), (28, 2),
          (30, 1), (31, 1))


def _build_tables(rows, cols, edge_mask, node_mask):
    """Permute nodes by degree, deal tiles round-robin to cores, and build the
    per-core padded gather tables (int16 indices wrapped the way
    InstDMAGatherAnt wants them) plus matching weight tables."""
    deg = np.bincount(rows, minlength=N)
    order = np.argsort(-deg, kind="stable")
    # global tile j -> core j%C, slot j//C ; permuted position of its p-th node
    perm = np.empty(N, dtype=np.int64)
    j = np.arange(N) // P                     # global tile of sorted rank r
    c = j % C
    t = j // C
    p = np.arange(N) % P
    perm[c * NL + t * P + p] = order          # perm[g] = original node id
    pos = np.empty(N, dtype=np.int64)
    pos[perm] = np.arange(N)

    # gather-table row id for permuted position g=(c,t,p):
    #   AllGather concatenates per-core [P, T*D] blocks, so
    #   row_id = c*NL + p*T + t
    gg = np.arange(N)
    gc, gr = gg // NL, gg % NL
    gt, gp_ = gr // P, gr % P
    rowid = gc * NL + gp_ * T + gt            # [g] -> table row
    dstpos = pos[rows]
    eorder = np.argsort(dstpos, kind="stable")
    src_sorted = rowid[pos[cols[eorder]]]     # gather table rows, 0..N-1
    w_sorted = edge_mask[eorder, 0].astype(np.float64)
    cnts = np.bincount(dstpos, minlength=N)
    offs = np.zeros(N + 1, dtype=np.int64)
    np.cumsum(cnts, out=offs[1:])

    # per-slot K: max count over the 8 cores' tiles in that slot
    cnts_g = cnts.reshape(C, T, P)
    Ks = np.maximum(cnts_g.max(axis=(0, 2)), 1).astype(np.int64)   # [T]

    IDXC = int(8 * Ks.sum())
    WTC = int(Ks.sum())
    idx_dev = np.zeros((C, P, IDXC), np.int16)
    wt_dev = np.zeros((C, P, WTC), np.float32)
    nm = node_mask[:, 0].astype(np.float64)
    ioff = woff = 0
    ar = None
    for t in range(T):
        K = int(Ks[t])
        if ar is None or ar.shape[1] != K:
            ar = np.arange(K)[None, :]
        for cc in range(C):
            base = cc * NL + t * P
            cn = cnts[base:base + P]
            take = offs[base:base + P][:, None] + ar          # [P, K]
            valid = ar < cn[:, None]
            take_c = np.minimum(take, E - 1)
            nb = np.where(valid, src_sorted[take_c], 0)
            wl = np.where(valid, w_sorted[take_c], 0.0)
            wl = wl * nm[perm[base:base + P]][:, None]
            il = nb.T.reshape(-1)                             # i = g*128+p
            ch = il.reshape(8 * K, 16).T                      # [16, 8K]
            idx_dev[cc, :, ioff:ioff + 8 * K] = np.tile(ch, (8, 1)).astype(np.int16)
            wt_dev[cc, :, woff:woff + K] = wl.astype(np.float32)
        ioff += 8 * K
        woff += K
    # transposed pad counts per chunk for the block-diagonal pad-subtract:
    # pcT[c][w, ci*P + p] = #pad entries of tile (lo_ci + w) partition p
    NCH = len(CHUNKS)
    pc_dev = np.zeros((C, 8, NCH * P), np.float32)
    for ci, (lo, n) in enumerate(CHUNKS):
        for w in range(n):
            t = lo + w
            K = int(Ks[t])
            for cc in range(C):
                base = cc * NL + t * P
                pc_dev[cc, w, ci * P:(ci + 1) * P] = K - cnts[base:base + P]
    allones = bool(np.all(edge_mask == 1.0) and np.all(node_mask == 1.0))
    return perm, Ks, idx_dev, wt_dev, IDXC, WTC, pc_dev, allones


def _build_program(Ks, IDXC, WTC, use_wt=True, sim=False):
    import concourse.bacc as bacc
    import concourse.bass as bass
    import concourse.mybir as mybir
    import concourse.tile as tile
    from concourse import library_config
    from concourse.masks import make_identity

    f32 = mybir.dt.float32
    bf16 = mybir.dt.bfloat16
    i16 = mybir.dt.int16
    AF = mybir.ActivationFunctionType
    OP = mybir.AluOpType
    X = mybir.AxisListType.X

    nc = bacc.Bacc("TRN2", target_bir_lowering=False, debug=False,
                   num_devices=1 if sim else C)

    h_in = nc.dram_tensor("h_in", [P, T * D], f32, kind="ExternalInput")
    hT_in = nc.dram_tensor("hT_in", [D, T * P], f32, kind="ExternalInput")
    idx_in = nc.dram_tensor("idx_in", [P, IDXC], i16, kind="ExternalInput")
    wt_in = nc.dram_tensor("wt_in", [P, WTC], f32, kind="ExternalInput")
    w0t_in = nc.dram_tensor("w0t_in", [D, D], f32, kind="ExternalInput")
    w1t_in = nc.dram_tensor("w1t_in", [D, D], f32, kind="ExternalInput")
    wot_in = nc.dram_tensor("wot_in", [D, 16], f32, kind="ExternalInput")
    pc_in = nc.dram_tensor("pc_in", [8, len(CHUNKS) * P], f32,
                           kind="ExternalInput")
    out_dram = nc.dram_tensor("out", [P, T * 16], f32, kind="ExternalOutput")
    xt_shard = nc.dram_tensor("xt_shard", [P, T * D], f32)
    xt_table = nc.dram_tensor("xt_table", [N, D], f32, addr_space="Shared")
    groups = [list(range(C))]

    ioffs = np.zeros(T + 1, dtype=np.int64)
    np.cumsum(8 * Ks, out=ioffs[1:])
    woffs = np.zeros(T + 1, dtype=np.int64)
    np.cumsum(Ks, out=woffs[1:])

    with tile.TileContext(nc) as tc:
        nc.gpsimd.load_library(library_config.mlp)
        import contextlib
        ctx = contextlib.ExitStack()
        with ctx:
            const = ctx.enter_context(tc.tile_pool(name="const", bufs=1))
            xtp = ctx.enter_context(tc.tile_pool(name="xtp", bufs=3))
            gp = ctx.enter_context(tc.tile_pool(name="gp", bufs=6))
            scp = ctx.enter_context(tc.tile_pool(name="scp", bufs=2))
            psp = ctx.enter_context(tc.tile_pool(name="psp", bufs=2, space="PSUM"))
            psmv = ctx.enter_context(tc.tile_pool(name="psmv", bufs=4, space="PSUM"))
            psc = ctx.enter_context(tc.tile_pool(name="psc", bufs=2, space="PSUM"))

            # ---- inputs ------------------------------------------------------
            x_sb = const.tile([P, T * D], f32)      # h (layer-0 input)
            nc.sync.dma_start(out=x_sb[:], in_=h_in[:])
            hT_sb = const.tile([D, T * P], f32)    # host-transposed h
            nc.sync.dma_start(out=hT_sb[:], in_=hT_in[:])
            w0t_sb = const.tile([D, D], f32)
            nc.sync.dma_start(out=w0t_sb[:], in_=w0t_in[:])
            w1t_sb = const.tile([D, D], f32)
            nc.sync.dma_start(out=w1t_sb[:], in_=w1t_in[:])
            wot_sb = const.tile([D, 16], f32)
            nc.sync.dma_start(out=wot_sb[:], in_=wot_in[:])
            idx_sb = const.tile([P, IDXC], i16)
            nc.sync.dma_start(out=idx_sb[:], in_=idx_in[:])
            if use_wt:
                wt_sb = const.tile([P, WTC], f32)
                nc.sync.dma_start(out=wt_sb[:], in_=wt_in[:])
            else:
                pc_sb = const.tile([8, len(CHUNKS) * P], f32)
                nc.sync.dma_start(out=pc_sb[:], in_=pc_in[:])
                # block-diagonal [8, 8*D]: row w holds row0 at cols w*D..;
                # the diagonal blocks are re-DMA'd from each layer's table
                br0_sb = const.tile([8, 8 * D], f32)
                nc.vector.memset(br0_sb[:], 0.0)

            ident = const.tile([P, P], f32)
            make_identity(nc, ident[:])

            mv_sb = const.tile([P, T * D], f32)     # raw W^T u, then messages
            agg_sb = const.tile([P, T * D], f32)    # aggregated tangent
            u_sb = const.tile([P, T * D], f32)      # relu'd tangent
            sq_sb = const.tile([P, T * D], f32)     # squares scratch
            out_sb = const.tile([P, T * 16], f32)
            nu2_all = const.tile([P, T], f32)       # |u|^2 for batched s3

            def ts(t, w=D):
                return slice(t * w, (t + 1) * w)

            def cs(lo, n, w=D):
                return slice(lo * w, (lo + n) * w)

            # ---- scalar-chain helpers (Rsqrt only => one act table) ---------
            # Key identities: artanh(min(tanh(min(x,15)), MAXN)) = min(x,
            # ARTMAXN), so logmap0(proj(expmap0(.))) collapses to the scale
            # min(1, ARTMAXN/|x|) = min(1, ARTMAXN*rsqrt(|x|^2)), and the full
            # matvec message scale to min(1, ART*rsqrt(|u|^2),
            # ART*rsqrt(|Wu|^2)).  Only the readout needs a real tanh (batched
            # once at the end).  ARTMAXN folds into Rsqrt's input scale.
            def rsq(dst, sq):
                """dst = ARTMAXN * rsqrt(max(sq, tiny)), in place allowed.
                sqrt(x/ART^2) = sqrt(x)/ART, then DVE reciprocal."""
                nc.vector.tensor_scalar_max(dst[:], sq[:], 1e-30)
                nc.scalar.activation(dst[:], dst[:], AF.Sqrt, scale=IART2)
                nc.vector.reciprocal(dst[:], dst[:])

            def rsq_min1(dst, sq):
                """dst = min(ARTMAXN * rsqrt(sq), 1)."""
                rsq(dst, sq)
                nc.vector.tensor_scalar_min(dst[:], dst[:], 1.0)

            def bcast(ap, w=D):
                """[P, W] -> [P, W, w] stride-0 broadcast view."""
                return bass.AP(ap.tensor, ap.offset, list(ap.ap) + [[0, w]])

            def sqreduce(dst, src_cols, eng=None):
                """dst[P, w] = per-tile sum of squares already in sq_sb cols."""
                (eng or nc.vector).tensor_reduce(
                    dst[:], sq_sb[:, src_cols].rearrange(
                        "p (w d) -> p w d", d=D), axis=X, op=OP.add)

            def matvec(src, w_l, tlist):
                """Per tile: transpose src tile, matmul with w_l into PSUM,
                square into sq_sb (Act), copy raw product to mv_sb (DVE)."""
                for t in tlist:
                    xT_ps = psp.tile([D, P], f32, tag="xT")
                    nc.tensor.transpose(out=xT_ps[:], in_=src[:, ts(t)],
                                        identity=ident[:])
                    xT = xtp.tile([D, P], f32, tag="xT_sb")
                    nc.scalar.activation(xT[:], xT_ps[:], AF.Copy)
                    mv_ps = psmv.tile([P, D], f32, tag="mv")
                    nc.tensor.matmul(out=mv_ps[:], lhsT=xT[:], rhs=w_l[:],
                                     start=True, stop=True)
                    nc.scalar.activation(sq_sb[:, ts(t)], mv_ps[:], AF.Square)
                    # alternate copy engine: both Act and DVE have slack
                    if t % 2:
                        nc.vector.tensor_copy(mv_sb[:, ts(t)], mv_ps[:])
                    else:
                        nc.scalar.activation(mv_sb[:, ts(t)], mv_ps[:],
                                             AF.Copy)

            def msg_scale(r1_ap, cols, w, tag, eng=None):
                """scl = min(r1, ART*rsqrt(|mv|^2)) applied to mv_sb[:, cols];
                r1_ap = min(ART*rsqrt(|x|^2), 1) from the layer input, mv
                squares already in sq_sb[:, cols]. eng picks the engine for
                the reduce and the big multiply (Pool only when no gather
                stream is active — it must never block desc-gen)."""
                m2 = scp.tile([P, w], f32, tag=f"m2{tag}")
                sqreduce(m2, cols)
                rsq(m2, m2)
                nc.vector.tensor_tensor(m2[:], m2[:], r1_ap, op=OP.min)
                mv3 = mv_sb[:, cols].rearrange("p (w d) -> p w d", d=D)
                (eng or nc.vector).tensor_tensor(mv3, mv3, bcast(m2[:]),
                                                 op=OP.mult)

            def allgather():
                tc.strict_bb_all_engine_barrier()
                if sim:
                    nc.sync.dma_start(
                        out=xt_table[0:NL, :].rearrange("(p x) d -> p x d", p=P),
                        in_=xt_shard[:].rearrange("p (x d) -> p x d", d=D))
                else:
                    nc.gpsimd.collective_compute(
                        "AllGather", mybir.AluOpType.bypass,
                        replica_groups=groups,
                        ins=[xt_shard[:, :]], outs=[xt_table[:, :]])
                tc.strict_bb_all_engine_barrier()

            # ---- head: layer-0 messages straight from h ---------------------
            # x0 = proj(expmap0(h)); msg = logmap0(proj(mobius_matvec(W0,x0)))
            #     = scl * (W^T h) with scl = min(1, ART*rsqrt(|h|^2),
            # ART*rsqrt(|Wh|^2)); host supplies hT so no transposes needed.
            # Grouped by 8 tiles so scale/publish of group g overlaps the
            # matmuls of group g+1; mv squares on Act, copies on DVE.
            for g0 in range(0, T, 8):
                gcols = cs(g0, 8)
                nc.scalar.activation(sq_sb[:, gcols], x_sb[:, gcols],
                                     AF.Square)
                n2g = scp.tile([P, 8], f32, tag="n2h")
                sqreduce(n2g, gcols)
                rsq_min1(n2g, n2g)
                for t in range(g0, g0 + 8):
                    mv_ps = psmv.tile([P, D], f32, tag="mv")
                    nc.tensor.matmul(out=mv_ps[:], lhsT=hT_sb[:, ts(t, P)],
                                     rhs=w0t_sb[:], start=True, stop=True)
                    nc.scalar.activation(sq_sb[:, ts(t)], mv_ps[:], AF.Square)
                    # alternate copy engine: both Act and DVE have slack
                    if t % 2:
                        nc.vector.tensor_copy(mv_sb[:, ts(t)], mv_ps[:])
                    else:
                        nc.scalar.activation(mv_sb[:, ts(t)], mv_ps[:],
                                             AF.Copy)
                msg_scale(n2g[:], gcols, 8, "h", nc.gpsimd)
                nc.sync.dma_start(out=xt_shard[:, gcols],
                                  in_=mv_sb[:, gcols])
            allgather()

            # ---- layers: gather stream with chunked trailing compute --------
            for layer in range(2):
                if not use_wt:
                    for w in range(8):
                        nc.sync.dma_start(
                            out=br0_sb[w:w + 1, w * D:(w + 1) * D],
                            in_=xt_table[0:1, :])
                for ci, (lo, nch) in enumerate(CHUNKS):
                    chunk = list(range(lo, lo + nch))
                    if not use_wt:
                        corr_ps = psc.tile([P, nch * D], f32, tag="corr")
                        nc.tensor.matmul(
                            out=corr_ps[:],
                            lhsT=pc_sb[:, ci * P:(ci + 1) * P],
                            rhs=br0_sb[:, 0:nch * D], start=True, stop=True)
                    for t in chunk:
                        K = int(Ks[t])
                        io, wo = int(ioffs[t]), int(woffs[t])
                        g = gp.tile([P, K * D], f32, tag="G")
                        g3 = g[:].rearrange("p (k d) -> p k d", d=D)
                        nc.gpsimd.dma_gather(
                            g3, xt_table[:, :], idx_sb[:, io:io + 8 * K],
                            num_idxs=P * K, num_idxs_reg=P * K, elem_size=D,
                            single_packet=False)
                        if use_wt:
                            wt_ap = wt_sb[:, wo:wo + K]
                            nc.vector.tensor_tensor(g3, g3, bcast(wt_ap),
                                                    op=OP.mult)
                        nc.vector.tensor_reduce(
                            agg_sb[:, ts(t)],
                            g[:].rearrange("p (k d) -> p d k", d=D),
                            axis=X, op=OP.add)
                    if not use_wt:
                        nc.vector.tensor_tensor(
                            agg_sb[:, cs(lo, nch)], agg_sb[:, cs(lo, nch)],
                            corr_ps[:], op=OP.subtract)
                    # chunk chain: u = relu(s2*agg),
                    # s2 = min(ART*rsqrt(|agg|^2), 1)  [collapsed identity]
                    ccols = cs(lo, nch)
                    nc.scalar.activation(sq_sb[:, ccols], agg_sb[:, ccols],
                                         AF.Square)
                    s2 = scp.tile([P, nch], f32, tag="s2c")
                    sqreduce(s2, ccols)
                    rsq_min1(s2, s2)
                    u3 = u_sb[:, ccols].rearrange("p (w d) -> p w d", d=D)
                    nc.vector.tensor_tensor(
                        u3, agg_sb[:, ccols].rearrange("p (w d) -> p w d", d=D),
                        bcast(s2[:]), op=OP.mult)
                    nc.scalar.activation(u_sb[:, ccols], u_sb[:, ccols],
                                         AF.Relu)
                    nc.scalar.activation(sq_sb[:, ccols], u_sb[:, ccols],
                                         AF.Square)
                    if layer == 0:
                        r1 = scp.tile([P, nch], f32, tag="r1c")
                        sqreduce(r1, ccols)
                        rsq_min1(r1, r1)
                        # next layer's raw matvec + message scale, publish
                        matvec(u_sb, w1t_sb, chunk)
                        msg_scale(r1[:], ccols, nch, "c")
                        nc.sync.dma_start(out=xt_shard[:, ccols],
                                          in_=mv_sb[:, ccols])
                    else:
                        # readout: raw u @ W_out^T now; the per-node scale
                        # s3 = min(tanh(|u|),MAXN)/|u| is batched at the end
                        # (the only real tanh -> one table switch total)
                        nc.vector.tensor_reduce(
                            nu2_all[:, lo:lo + nch],
                            sq_sb[:, ccols].rearrange("p (w d) -> p w d", d=D),
                            axis=X, op=OP.add)
                        for t in chunk:
                            xT_ps = psp.tile([D, P], f32, tag="xT")
                            nc.tensor.transpose(out=xT_ps[:],
                                                in_=u_sb[:, ts(t)],
                                                identity=ident[:])
                            xT = xtp.tile([D, P], f32, tag="xT_sb")
                            nc.scalar.activation(xT[:], xT_ps[:], AF.Copy)
                            o_ps = psmv.tile([P, 16], f32, tag="mv")
                            nc.tensor.matmul(out=o_ps[:], lhsT=xT[:],
                                             rhs=wot_sb[:], start=True,
                                             stop=True)
                            nc.scalar.activation(out_sb[:, ts(t, 16)], o_ps[:],
                                                 AF.Copy)
                if layer == 0:
                    allgather()

            # batched readout scale: s3 = min(tanh(min(nu,15)), MAXN)/nu with
            # tanh = 1-2/(1+e^2x); rr = rsqrt(nu2), nu = nu2*rr, s3 = a3*rr
            rr = scp.tile([P, T], f32, tag="rr")
            nc.vector.tensor_scalar_max(rr[:], nu2_all[:], 1e-30)
            nc.scalar.activation(rr[:], rr[:], AF.Sqrt)
            nc.vector.reciprocal(rr[:], rr[:])
            a3 = scp.tile([P, T], f32, tag="a3")
            nc.vector.tensor_tensor(a3[:], nu2_all[:], rr[:], op=OP.mult)
            nc.vector.tensor_scalar_min(a3[:], a3[:], MAX_TANH)
            nc.scalar.activation(a3[:], a3[:], AF.Exp, scale=2.0)
            nc.scalar.activation(a3[:], a3[:], AF.Copy, bias=1.0)
            nc.vector.reciprocal(a3[:], a3[:])
            nc.scalar.activation(a3[:], a3[:], AF.Copy, scale=-2.0, bias=1.0)
            nc.vector.tensor_scalar_min(a3[:], a3[:], MAXN)
            nc.vector.tensor_tensor(a3[:], a3[:], rr[:], op=OP.mult)
            out3 = out_sb[:].rearrange("p (t o) -> p t o", o=16)
            nc.vector.tensor_tensor(out3, out3, bcast(a3[:], w=16),
                                    op=OP.mult)
            nc.sync.dma_start(out=out_dram[:], in_=out_sb[:])
    nc.compile()
    return nc


def kernel(h, distances, rows, cols, node_mask, edge_mask,
           W0, b0, W1, b1, W_out, b_out, _trace=False):
    from concourse.bass_utils import run_bass_kernel_spmd

    h = np.asarray(h, dtype=np.float32)
    rows = np.asarray(rows).astype(np.int64)
    cols = np.asarray(cols).astype(np.int64)
    node_mask = np.asarray(node_mask, dtype=np.float32)
    edge_mask = np.asarray(edge_mask, dtype=np.float32)
    assert not np.any(np.asarray(b0)) and not np.any(np.asarray(b1)) and \
        not np.any(np.asarray(b_out)), "nonzero biases unsupported"

    perm, Ks, idx_dev, wt_dev, IDXC, WTC, pc_dev, allones = _build_tables(
        rows, cols, edge_mask, node_mask)

    hp = h[perm].reshape(C, T, P, D).transpose(0, 2, 1, 3).reshape(C, P, T * D)
    # hT[c][d, t*P+p] = h[node (c,t,p), d] for transpose-free head matmuls
    hT = h[perm].reshape(C, T, P, D).transpose(0, 3, 1, 2).reshape(C, D, T * P)
    w0t = np.ascontiguousarray(np.asarray(W0, np.float32).T)
    w1t = np.ascontiguousarray(np.asarray(W1, np.float32).T)
    wot = np.ascontiguousarray(np.asarray(W_out, np.float32).T)

    nc = _build_program(Ks, IDXC, WTC, use_wt=not allones)
    in_maps = [{
        "h_in": np.ascontiguousarray(hp[c]),
        "hT_in": np.ascontiguousarray(hT[c]),
        "idx_in": idx_dev[c],
        "wt_in": wt_dev[c],
        "w0t_in": w0t, "w1t_in": w1t, "wot_in": wot,
        "pc_in": pc_dev[c],
    } for c in range(C)]
    res = run_bass_kernel_spmd(nc, in_maps, list(range(C)), trace=_trace)
    od = np.stack([res.results[c]["out"] for c in range(C)])
    od = od.reshape(C, P, T, 16).transpose(0, 2, 1, 3).reshape(N, 16)
    out = np.empty((N, 16), np.float32)
    out[perm] = od
    if _trace:
        return out, res
    return out
